# revision 7
# baseline (speedup 1.0000x reference)
"""ALiBi attention (B=2, N=2048, C=1024, H=16, D=64) on 8 TRN2 NeuronCores.

Sharding: core i owns heads (2i, 2i+1) for both batches (4 [N,N] score blocks
per core). Q/K/V/first-proj are column-split over heads; output projection is
computed n-sharded after an AllToAll of the per-head attention outputs.

Precision: the reference DIVIDES by scale (multiplies scores by sqrt(D)=8), so
score noise from bf16 rounding of Q/K would be ~0.2 absolute. All matmuls that
feed scores therefore use an exact bf16 hi/lo split: main = hi*hi (exact in the
PE's fp32 accumulator) plus one stacked cross-term matmul (hi*lo + lo*hi).

Layouts (transposed activations, contraction on partitions):
  xT [C, B*N] -> qT/kT [e, n] per head; v natural [m, e].
  pass1 (row-max for softmax stability): S1[n, m] via lhsT=Q rhs=K, DVE
    reduce_max -> -M[n], folded back into Q's aug row via PE transpose + DMA.
  pass2: S2[m, n] = qk - slope*n - M[n] via aug rows; ACT exp adds +slope*m
    as per-partition fp32 bias; AV matmul with a ones-column in V gives the
    softmax denominator for free.
"""
import numpy as np
import ml_dtypes

import concourse.bacc as bacc
import concourse.mybir as mybir
import concourse.tile as tile
from concourse.bass_utils import run_bass_kernel_spmd

F32 = mybir.dt.float32
BF16 = mybir.dt.bfloat16
BF = ml_dtypes.bfloat16

B, N, C, H, D = 2, 2048, 1024, 16, 64
NCORES = 8
HL = H // NCORES          # heads per core (2)
BN = B * N                # 4096
NSH = BN // NCORES        # 512 output columns per core
CCH = C // 128            # 8 contraction chunks
NBH = B * HL              # 4 (batch, local-head) blocks per core
MC = N // 128             # 16 m-chunks per sequence
AX = mybir.AxisListType
ALU = mybir.AluOpType
ACT = mybir.ActivationFunctionType

_compiled = None


def _build():
    nc = bacc.Bacc("TRN2", target_bir_lowering=False, debug=False,
                   num_devices=NCORES)

    x_hi = nc.dram_tensor("x_hi", [128, CCH, BN], BF16, kind="ExternalInput")
    x_lo = nc.dram_tensor("x_lo", [128, CCH, BN], BF16, kind="ExternalInput")
    wq_hi = nc.dram_tensor("wq_hi", [128, CCH, 128], BF16, kind="ExternalInput")
    wq_lo = nc.dram_tensor("wq_lo", [128, CCH, 128], BF16, kind="ExternalInput")
    wk_hi = nc.dram_tensor("wk_hi", [128, CCH, 128], BF16, kind="ExternalInput")
    wk_lo = nc.dram_tensor("wk_lo", [128, CCH, 128], BF16, kind="ExternalInput")
    wv = nc.dram_tensor("wv", [128, CCH, 128], BF16, kind="ExternalInput")
    wp = nc.dram_tensor("wp", [128, CCH, C], BF16, kind="ExternalInput")
    bp_t = nc.dram_tensor("bp_t", [128, CCH], F32, kind="ExternalInput")
    qaug = nc.dram_tensor("qaug", [HL, 3, N], BF16, kind="ExternalInput")
    kaug = nc.dram_tensor("kaug", [HL, 3, N], BF16, kind="ExternalInput")
    mbias = nc.dram_tensor("mbias", [128, HL * MC], F32, kind="ExternalInput")
    ident = nc.dram_tensor("ident", [128, 128], F32, kind="ExternalInput")
    out_t = nc.dram_tensor("out", [C, NSH], F32, kind="ExternalOutput")

    with tile.TileContext(nc) as tc:
        with tc.tile_pool(name="wpool", bufs=1) as wpool, \
             tc.tile_pool(name="xpool", bufs=1) as xpool, \
             tc.tile_pool(name="qkpool", bufs=1) as qkpool, \
             tc.tile_pool(name="aux", bufs=2) as aux, \
             tc.tile_pool(name="attp", bufs=1) as attp, \
             tc.tile_pool(name="psum", bufs=1, space="PSUM") as psum, \
             tc.tile_pool(name="dram", bufs=1, space="DRAM") as dram:

            # ---------- resident weights / aux ----------
            wq_hi_sb = wpool.tile([128, CCH, 128], BF16)
            wq_lo_sb = wpool.tile([128, CCH, 128], BF16)
            wk_hi_sb = wpool.tile([128, CCH, 128], BF16)
            wk_lo_sb = wpool.tile([128, CCH, 128], BF16)
            wv_sb = wpool.tile([128, CCH, 128], BF16)
            wp_sb = wpool.tile([128, CCH, C], BF16)
            bp_sb = wpool.tile([128, CCH], F32)
            mbias_sb = wpool.tile([128, HL * MC], F32)
            ident_sb = wpool.tile([128, 128], F32)
            for sb_t, dr_t in ((wq_hi_sb, wq_hi), (wq_lo_sb, wq_lo),
                               (wk_hi_sb, wk_hi), (wk_lo_sb, wk_lo),
                               (wv_sb, wv), (wp_sb, wp), (bp_sb, bp_t),
                               (mbias_sb, mbias), (ident_sb, ident)):
                nc.sync.dma_start(sb_t[:], dr_t[:, :])

            # ---------- per-(batch, local-head) persistent tiles ----------
            QT, KT, QC, KC, VA, MP = [], [], [], [], [], []
            for i in range(NBH):
                j = i % HL
                q = qkpool.tile([67, N], BF16, name=f"Qt{i}", tag=f"Qt{i}")
                k = qkpool.tile([67, N], BF16, name=f"Kt{i}", tag=f"Kt{i}")
                qc = qkpool.tile([128, N], BF16, name=f"Qc{i}", tag=f"Qc{i}")
                kc = qkpool.tile([128, N], BF16, name=f"Kc{i}", tag=f"Kc{i}")
                va = qkpool.tile([128, MC, 65], BF16, name=f"Va{i}", tag=f"Va{i}")
                mp = qkpool.tile([128, 32], F32, name=f"Mp{i}", tag=f"Mp{i}")
                # q rows 64-66: [-slope*n; -M placeholder (0); ones]
                nc.sync.dma_start(q[64:67, :], qaug[j, :, :])
                # k rows 64-66: [ones; ones; +slope*m]
                nc.sync.dma_start(k[64:67, :], kaug[j, :, :])
                nc.any.memset(va[:, :, 64:65], 1.0)                # denominator ones
                QT.append(q); KT.append(k); QC.append(qc); KC.append(kc)
                VA.append(va); MP.append(mp)

            # ---------- projections (8 blocks of 512 over B*N) ----------
            for blk in range(CCH):
                b = blk // 4
                col0 = blk * 512
                nw = blk % 4  # n-block within batch
                xh, xl = [], []
                for c in range(CCH):
                    th = xpool.tile([128, 512], BF16, name=f"xh{blk}_{c}", tag="xh", bufs=16)
                    tl = xpool.tile([128, 512], BF16, name=f"xl{blk}_{c}", tag="xl", bufs=16)
                    nc.sync.dma_start(th[:], x_hi[:, c, col0:col0 + 512])
                    nc.sync.dma_start(tl[:], x_lo[:, c, col0:col0 + 512])
                    xh.append(th); xl.append(tl)

                for w_hi_t, w_lo_t, T, TC, hi_first in (
                        (wq_hi_sb, wq_lo_sb, QT, QC, True),
                        (wk_hi_sb, wk_lo_sb, KT, KC, False)):
                    ps = psum.tile([128, 512], F32, name=f"pj{blk}_{int(hi_first)}",
                                   tag="qs", bufs=2)
                    nmm = 3 * CCH
                    idx = 0
                    for c in range(CCH):
                        nc.tensor.matmul(ps[:], w_hi_t[:, c, :], xh[c][:],
                                         start=(idx == 0), stop=(idx == nmm - 1))
                        idx += 1
                    for c in range(CCH):
                        nc.tensor.matmul(ps[:], w_lo_t[:, c, :], xh[c][:],
                                         start=False, stop=(idx == nmm - 1))
                        idx += 1
                    for c in range(CCH):
                        nc.tensor.matmul(ps[:], w_hi_t[:, c, :], xl[c][:],
                                         start=False, stop=(idx == nmm - 1))
                        idx += 1
                    cols = slice(nw * 512, nw * 512 + 512)
                    for j in range(HL):
                        i = b * HL + j
                        rows = slice(64 * j, 64 * j + 64)
                        # hi part into the aug tile
                        nc.any.tensor_copy(T[i][0:64, cols], ps[rows, :])
                        if hi_first:   # Qc = [q_hi; q_lo]
                            nc.any.tensor_copy(TC[i][0:64, cols], T[i][0:64, cols])
                            nc.vector.tensor_sub(TC[i][64:128, cols], ps[rows, :],
                                                 T[i][0:64, cols])
                        else:          # Kc = [k_lo; k_hi]
                            nc.any.tensor_copy(TC[i][64:128, cols], T[i][0:64, cols])
                            nc.vector.tensor_sub(TC[i][0:64, cols], ps[rows, :],
                                                 T[i][0:64, cols])

                # v in natural [m, e] layout
                for mt in range(4):
                    vps = psum.tile([128, 128], F32, name=f"v{blk}_{mt}",
                                    tag="sm", bufs=2)
                    for c in range(CCH):
                        nc.tensor.matmul(vps[:], xh[c][:, mt * 128:(mt + 1) * 128],
                                         wv_sb[:, c, :],
                                         start=(c == 0), stop=(c == CCH - 1))
                    mc = nw * 4 + mt
                    for j in range(HL):
                        i = b * HL + j
                        nc.any.tensor_copy(VA[i][:, mc, 0:64],
                                           vps[:, 64 * j:64 * j + 64])

            # ---------- attention ----------
            ag_in = dram.tile([NCORES, 128, NSH], BF16)
            ag_out = dram.tile([NCORES, 128, NSH], BF16)

            def emit_pass1(i):
                Q, K, Mpt = QT[i], KT[i], MP[i]
                for nt in range(16):
                    for half in range(2):
                        ps = psum.tile([128, 1024], F32, tag="p1", bufs=2,
                                       name=f"p1_{i}_{nt}_{half}")
                        for mb in range(2):
                            m0 = (half * 2 + mb) * 512
                            nc.tensor.matmul(ps[:, mb * 512:(mb + 1) * 512],
                                             Q[0:67, nt * 128:(nt + 1) * 128],
                                             K[0:67, m0:m0 + 512],
                                             start=True, stop=True)
                        nc.vector.tensor_reduce(
                            Mpt[:, nt * 2 + half:nt * 2 + half + 1], ps[:, :],
                            axis=AX.X, op=ALU.max)
                mneg = aux.tile([128, 16], F32, tag="mneg", name=f"mneg{i}")
                nc.vector.tensor_reduce(
                    mneg[:], Mpt[:].rearrange("p (a b) -> p a b", b=2),
                    axis=AX.X, op=ALU.max, negate=True)
                trp = psum.tile([16, 128], F32, tag="sm", bufs=2, name=f"trp{i}")
                nc.tensor.transpose(trp[:], mneg[:], ident_sb[:])
                mrow16 = aux.tile([16, 128], BF16, tag="mrow16", name=f"mr{i}")
                nc.any.tensor_copy(mrow16[:], trp[:])
                nc.sync.dma_start(Q[65:66, :], mrow16[:, :])

            def emit_pass2(i):
                b, j = divmod(i, HL)
                Q, K, Qc, Kc, Va = QT[i], KT[i], QC[i], KC[i], VA[i]
                for nb in range(4):
                    n0 = nb * 512
                    avp = psum.tile([65, 512], F32, tag="sm", bufs=2,
                                    name=f"av_{i}_{nb}")
                    for mc in range(MC):
                        s2 = psum.tile([128, 512], F32, tag="qs", bufs=2,
                                       name=f"s2_{i}_{nb}_{mc}")
                        nc.tensor.matmul(s2[:], K[0:66, mc * 128:(mc + 1) * 128],
                                         Q[0:66, n0:n0 + 512],
                                         start=True, stop=False)
                        nc.tensor.matmul(s2[:], Kc[:, mc * 128:(mc + 1) * 128],
                                         Qc[:, n0:n0 + 512],
                                         start=False, stop=True)
                        at = attp.tile([128, 512], BF16, tag="att", bufs=4,
                                       name=f"at_{i}_{nb}_{mc}")
                        nc.scalar.activation(at[:], s2[:], ACT.Exp,
                                             bias=mbias_sb[:, j * MC + mc:j * MC + mc + 1],
                                             scale=1.0)
                        nc.tensor.matmul(avp[:], Va[:, mc, :], at[:],
                                         start=(mc == 0), stop=(mc == MC - 1))
                    linv = aux.tile([1, 512], F32, tag="linv", name=f"li_{i}_{nb}")
                    nc.vector.reciprocal(linv[0:1, :], avp[64:65, :])
                    lb = aux.tile([64, 512], F32, tag="lb", name=f"lb_{i}_{nb}")
                    nc.gpsimd.partition_broadcast(lb[:], linv[0:1, :])
                    gt = aux.tile([64, 512], BF16, tag="gt", name=f"gt_{i}_{nb}")
                    nc.vector.tensor_mul(gt[:], avp[0:64, :], lb[:])
                    s = b * 4 + nb
                    nc.sync.dma_start(ag_in[s, 64 * j:64 * j + 64, :], gt[:])

            emit_pass1(0)
            for i in range(1, NBH):
                emit_pass1(i)
                emit_pass2(i - 1)
            emit_pass2(NBH - 1)

            # ---------- collective + output projection ----------
            nc.gpsimd.collective_compute(
                "AllToAll", ALU.bypass,
                replica_groups=[list(range(NCORES))],
                ins=[ag_in.opt()],
                outs=[ag_out.opt()],
            )
            gt_in = attp.tile([128, CCH, NSH], BF16, tag="gtin", bufs=1)
            nc.sync.dma_start(gt_in[:, :, :],
                              ag_out[:, :, :].rearrange("c p f -> p c f"))
            for et in range(CCH):
                yps = psum.tile([128, 512], F32, tag="qs", bufs=2, name=f"y{et}")
                for c in range(CCH):
                    nc.tensor.matmul(yps[:], wp_sb[:, c, et * 128:(et + 1) * 128],
                                     gt_in[:, c, :],
                                     start=(c == 0), stop=(c == CCH - 1))
                ysb = aux.tile([128, 512], F32, tag="y", name=f"ysb{et}")
                nc.scalar.activation(ysb[:], yps[:], ACT.Identity,
                                     bias=bp_sb[:, et:et + 1], scale=1.0)
                nc.sync.dma_start(out_t[et * 128:(et + 1) * 128, :], ysb[:])

    nc.compile()
    return nc


def _get_nc():
    global _compiled
    if _compiled is None:
        _compiled = _build()
    return _compiled


def _alibi_slopes():
    x = (2 ** 8) ** (1.0 / H)
    return np.array([1.0 / x ** (i + 1) for i in range(H)], dtype=np.float64)


def _chunked(a):
    """[C, F] -> [128, CCH, F] (partition, c-chunk, free)."""
    Cdim, F = a.shape
    return np.ascontiguousarray(a.reshape(CCH, 128, F).transpose(1, 0, 2))


def _split(a):
    hi = a.astype(BF)
    lo = (a - hi.astype(np.float32)).astype(BF)
    return hi, lo


def _make_in_maps(x, Wq, Wk, Wv, Wp, bp):
    x = np.asarray(x, dtype=np.float32)
    xT = np.ascontiguousarray(x.reshape(BN, C).T)          # [C, BN]
    xch = _chunked(xT)
    xch_hi, xch_lo = _split(xch)

    slopes = _alibi_slopes()
    n_arr = np.arange(N, dtype=np.float64)
    p_arr = np.arange(128, dtype=np.float64)

    wp_ch = _chunked(np.ascontiguousarray(np.asarray(Wp, np.float32).T)).astype(BF)
    bp_tile = np.ascontiguousarray(
        np.asarray(bp, np.float32).reshape(CCH, 128).T)
    identity = np.eye(128, dtype=np.float32)

    in_maps = []
    for core in range(NCORES):
        e0 = core * 128
        wqT = np.ascontiguousarray((8.0 * np.asarray(Wq, np.float32)[e0:e0 + 128]).T)
        wkT = np.ascontiguousarray(np.asarray(Wk, np.float32)[e0:e0 + 128].T)
        wvT = np.ascontiguousarray(np.asarray(Wv, np.float32)[e0:e0 + 128].T)
        wq_h, wq_l = _split(_chunked(wqT))
        wk_h, wk_l = _split(_chunked(wkT))

        s = slopes[core * HL: core * HL + HL]               # [HL]
        qaug = np.zeros((HL, 3, N), dtype=BF)
        kaug = np.zeros((HL, 3, N), dtype=BF)
        for j in range(HL):
            qaug[j, 0] = (-s[j] * n_arr).astype(BF)   # -slope*n
            qaug[j, 1] = 0.0                          # -M placeholder
            qaug[j, 2] = 1.0
            kaug[j, 0] = 1.0
            kaug[j, 1] = 1.0
            kaug[j, 2] = (s[j] * n_arr).astype(BF)    # +slope*m
        mb = np.zeros((128, HL * MC), dtype=np.float32)
        for j in range(HL):
            for c in range(MC):
                mb[:, j * MC + c] = s[j] * (128 * c + p_arr)

        in_maps.append({
            "x_hi": xch_hi, "x_lo": xch_lo,
            "wq_hi": wq_h, "wq_lo": wq_l,
            "wk_hi": wk_h, "wk_lo": wk_l,
            "wv": _chunked(wvT).astype(BF),
            "wp": wp_ch, "bp_t": bp_tile,
            "qaug": qaug, "kaug": kaug, "mbias": mb,
            "ident": identity,
        })
    return in_maps


def run(x, Wq, Wk, Wv, Wp, bp, trace=False, tmpdir=None):
    nc = _get_nc()
    in_maps = _make_in_maps(x, Wq, Wk, Wv, Wp, bp)
    kwargs = {}
    if trace:
        kwargs = {"trace": True, "tmpdir": tmpdir}
    res = run_bass_kernel_spmd(nc, in_maps, core_ids=list(range(NCORES)), **kwargs)
    yT = np.concatenate([res.results[i]["out"] for i in range(NCORES)], axis=1)
    out = np.ascontiguousarray(yT.T).reshape(B, N, C).astype(np.float32)
    return out, res


def kernel(x, Wq, Wk, Wv, Wp, bp):
    out, _ = run(x, Wq, Wk, Wv, Wp, bp)
    return out


# revision 14
# speedup vs baseline: 1.0103x; 1.0103x over previous
"""ALiBi attention (B=2, N=2048, C=1024, H=16, D=64) on 8 TRN2 NeuronCores.

Sharding: core i owns heads (2i, 2i+1) for both batches (4 [N,N] score blocks
per core). Q/K/V/first-proj are column-split over heads; output projection is
computed n-sharded after an AllToAll of the per-head attention outputs.

Precision: the reference DIVIDES by scale (multiplies scores by sqrt(D)=8), so
score noise from bf16 rounding of Q/K would be ~0.2 absolute. All matmuls that
feed scores therefore use an exact bf16 hi/lo split: main = hi*hi (exact in the
PE's fp32 accumulator) plus one stacked cross-term matmul (hi*lo + lo*hi).

Layouts (transposed activations, contraction on partitions):
  xT [C, B*N] -> qT/kT [e, n] per head; v natural [m, e].
  pass1 (row-max for softmax stability): S1[n, m] via lhsT=Q rhs=K, DVE
    reduce_max -> -M[n], folded back into Q's aug row via PE transpose + DMA.
  pass2: S2[m, n] = qk - slope*n - M[n] via aug rows; ACT exp adds +slope*m
    as per-partition fp32 bias; AV matmul with a ones-column in V gives the
    softmax denominator for free.
"""
import numpy as np
import ml_dtypes

import concourse.bacc as bacc
import concourse.mybir as mybir
import concourse.tile as tile
from concourse.bass_utils import run_bass_kernel_spmd

F32 = mybir.dt.float32
BF16 = mybir.dt.bfloat16
BF = ml_dtypes.bfloat16

B, N, C, H, D = 2, 2048, 1024, 16, 64
NCORES = 8
HL = H // NCORES          # heads per core (2)
BN = B * N                # 4096
NSH = BN // NCORES        # 512 output columns per core
CCH = C // 128            # 8 contraction chunks
NBH = B * HL              # 4 (batch, local-head) blocks per core
MC = N // 128             # 16 m-chunks per sequence
AX = mybir.AxisListType
ALU = mybir.AluOpType
ACT = mybir.ActivationFunctionType

_compiled = None


def _build():
    nc = bacc.Bacc("TRN2", target_bir_lowering=False, debug=False,
                   num_devices=NCORES)

    x_hi = nc.dram_tensor("x_hi", [128, CCH, BN], BF16, kind="ExternalInput")
    x_lo = nc.dram_tensor("x_lo", [128, CCH, BN], BF16, kind="ExternalInput")
    wq_hi = nc.dram_tensor("wq_hi", [128, CCH, 128], BF16, kind="ExternalInput")
    wq_lo = nc.dram_tensor("wq_lo", [128, CCH, 128], BF16, kind="ExternalInput")
    wk_hi = nc.dram_tensor("wk_hi", [128, CCH, 128], BF16, kind="ExternalInput")
    wk_lo = nc.dram_tensor("wk_lo", [128, CCH, 128], BF16, kind="ExternalInput")
    wv = nc.dram_tensor("wv", [128, CCH, 128], BF16, kind="ExternalInput")
    wp = nc.dram_tensor("wp", [128, CCH, C], BF16, kind="ExternalInput")
    bp_t = nc.dram_tensor("bp_t", [128, CCH], F32, kind="ExternalInput")
    qaug = nc.dram_tensor("qaug", [HL, 3, N], BF16, kind="ExternalInput")
    kaug = nc.dram_tensor("kaug", [HL, 3, N], BF16, kind="ExternalInput")
    mbias = nc.dram_tensor("mbias", [128, HL * MC], F32, kind="ExternalInput")
    ident = nc.dram_tensor("ident", [128, 128], F32, kind="ExternalInput")
    out_t = nc.dram_tensor("out", [C, NSH], F32, kind="ExternalOutput")

    with tile.TileContext(nc) as tc:
        with tc.tile_pool(name="wpool", bufs=1) as wpool, \
             tc.tile_pool(name="xpool", bufs=1) as xpool, \
             tc.tile_pool(name="qkpool", bufs=1) as qkpool, \
             tc.tile_pool(name="aux", bufs=2) as aux, \
             tc.tile_pool(name="attp", bufs=1) as attp, \
             tc.tile_pool(name="psum", bufs=1, space="PSUM") as psum, \
             tc.tile_pool(name="dram", bufs=1, space="DRAM") as dram:

            # ---------- resident weights / aux ----------
            wq_hi_sb = wpool.tile([128, CCH, 128], BF16)
            wq_lo_sb = wpool.tile([128, CCH, 128], BF16)
            wk_hi_sb = wpool.tile([128, CCH, 128], BF16)
            wk_lo_sb = wpool.tile([128, CCH, 128], BF16)
            wv_sb = wpool.tile([128, CCH, 128], BF16)
            wp_sb = wpool.tile([128, CCH, C], BF16)
            bp_sb = wpool.tile([128, CCH], F32)
            mbias_sb = wpool.tile([128, HL * MC], F32)
            ident_sb = wpool.tile([128, 128], F32)
            for sb_t, dr_t in ((wq_hi_sb, wq_hi), (wq_lo_sb, wq_lo),
                               (wk_hi_sb, wk_hi), (wk_lo_sb, wk_lo),
                               (wv_sb, wv), (wp_sb, wp), (bp_sb, bp_t),
                               (mbias_sb, mbias), (ident_sb, ident)):
                nc.sync.dma_start(sb_t[:], dr_t[:, :])

            # ---------- per-(batch, local-head) persistent tiles ----------
            QT, KT, QC, KC, VA, MP = [], [], [], [], [], []
            for i in range(NBH):
                j = i % HL
                # full 128 partitions; rows 67-127 zeroed so score matmuls can
                # run K=128 (K<=66 keeps the PE clock-gated at 1.2 GHz)
                q = qkpool.tile([128, N], BF16, name=f"Qt{i}", tag=f"Qt{i}")
                k = qkpool.tile([128, N], BF16, name=f"Kt{i}", tag=f"Kt{i}")
                qc = qkpool.tile([128, N], BF16, name=f"Qc{i}", tag=f"Qc{i}")
                kc = qkpool.tile([128, N], BF16, name=f"Kc{i}", tag=f"Kc{i}")
                va = qkpool.tile([128, MC, 65], BF16, name=f"Va{i}", tag=f"Va{i}")
                mp = qkpool.tile([128, 32], F32, name=f"Mp{i}", tag=f"Mp{i}")
                nc.any.memset(q[64:128, :], 0.0)
                nc.any.memset(k[64:128, :], 0.0)
                # q rows 64-66: [-slope*n; -M placeholder (0); ones]
                nc.sync.dma_start(q[64:67, :], qaug[j, :, :])
                # k rows 64-66: [ones; ones; +slope*m]
                nc.sync.dma_start(k[64:67, :], kaug[j, :, :])
                nc.any.memset(va[:, :, 64:65], 1.0)                # denominator ones
                QT.append(q); KT.append(k); QC.append(qc); KC.append(kc)
                VA.append(va); MP.append(mp)

            # ---------- projections (8 blocks of 512 over B*N) ----------
            for blk in range(CCH):
                b = blk // 4
                col0 = blk * 512
                nw = blk % 4  # n-block within batch
                xh, xl = [], []
                for c in range(CCH):
                    th = xpool.tile([128, 512], BF16, name=f"xh{blk}_{c}", tag="xh", bufs=16)
                    tl = xpool.tile([128, 512], BF16, name=f"xl{blk}_{c}", tag="xl", bufs=16)
                    nc.sync.dma_start(th[:], x_hi[:, c, col0:col0 + 512])
                    nc.sync.dma_start(tl[:], x_lo[:, c, col0:col0 + 512])
                    xh.append(th); xl.append(tl)

                for w_hi_t, w_lo_t, T, TC, hi_first in (
                        (wq_hi_sb, wq_lo_sb, QT, QC, True),
                        (wk_hi_sb, wk_lo_sb, KT, KC, False)):
                    ps = psum.tile([128, 512], F32, name=f"pj{blk}_{int(hi_first)}",
                                   tag="qs", bufs=2)
                    nmm = 3 * CCH
                    idx = 0
                    for c in range(CCH):
                        nc.tensor.matmul(ps[:], w_hi_t[:, c, :], xh[c][:],
                                         start=(idx == 0), stop=(idx == nmm - 1))
                        idx += 1
                    for c in range(CCH):
                        nc.tensor.matmul(ps[:], w_lo_t[:, c, :], xh[c][:],
                                         start=False, stop=(idx == nmm - 1))
                        idx += 1
                    for c in range(CCH):
                        nc.tensor.matmul(ps[:], w_hi_t[:, c, :], xl[c][:],
                                         start=False, stop=(idx == nmm - 1))
                        idx += 1
                    cols = slice(nw * 512, nw * 512 + 512)
                    for j in range(HL):
                        i = b * HL + j
                        rows = slice(64 * j, 64 * j + 64)
                        # hi part into the aug tile
                        nc.any.tensor_copy(T[i][0:64, cols], ps[rows, :])
                        if hi_first:   # Qc = [q_hi; q_lo]
                            nc.any.tensor_copy(TC[i][0:64, cols], T[i][0:64, cols])
                            nc.vector.tensor_sub(TC[i][64:128, cols], ps[rows, :],
                                                 T[i][0:64, cols])
                        else:          # Kc = [k_lo; k_hi]
                            nc.any.tensor_copy(TC[i][64:128, cols], T[i][0:64, cols])
                            nc.vector.tensor_sub(TC[i][0:64, cols], ps[rows, :],
                                                 T[i][0:64, cols])

                # v in natural [m, e] layout
                for mt in range(4):
                    vps = psum.tile([128, 128], F32, name=f"v{blk}_{mt}",
                                    tag="sm", bufs=2)
                    for c in range(CCH):
                        nc.tensor.matmul(vps[:], xh[c][:, mt * 128:(mt + 1) * 128],
                                         wv_sb[:, c, :],
                                         start=(c == 0), stop=(c == CCH - 1))
                    mc = nw * 4 + mt
                    for j in range(HL):
                        i = b * HL + j
                        nc.any.tensor_copy(VA[i][:, mc, 0:64],
                                           vps[:, 64 * j:64 * j + 64])

            # ---------- attention ----------
            ag_in = dram.tile([NCORES, 128, NSH], BF16)
            ag_out = dram.tile([NCORES, 128, NSH], BF16)

            def emit_pass1(i):
                Q, K, Mpt = QT[i], KT[i], MP[i]
                for nt in range(16):
                    for half in range(2):
                        ps = psum.tile([128, 1024], F32, tag="p1", bufs=2,
                                       name=f"p1_{i}_{nt}_{half}")
                        for mb in range(2):
                            m0 = (half * 2 + mb) * 512
                            nc.tensor.matmul(ps[:, mb * 512:(mb + 1) * 512],
                                             Q[:, nt * 128:(nt + 1) * 128],
                                             K[:, m0:m0 + 512],
                                             start=True, stop=True)
                        nc.vector.tensor_reduce(
                            Mpt[:, nt * 2 + half:nt * 2 + half + 1], ps[:, :],
                            axis=AX.X, op=ALU.max)
                mneg = aux.tile([128, 16], F32, tag="mneg", name=f"mneg{i}")
                nc.vector.tensor_reduce(
                    mneg[:], Mpt[:].rearrange("p (a b) -> p a b", b=2),
                    axis=AX.X, op=ALU.max, negate=True)
                trp = psum.tile([16, 128], F32, tag="sm", bufs=2, name=f"trp{i}")
                nc.tensor.transpose(trp[:], mneg[:], ident_sb[:])
                mrow16 = aux.tile([16, 128], BF16, tag="mrow16", name=f"mr{i}")
                nc.any.tensor_copy(mrow16[:], trp[:])
                nc.sync.dma_start(Q[65:66, :], mrow16[:, :])

            def emit_pass2(i):
                b, j = divmod(i, HL)
                Q, K, Qc, Kc, Va = QT[i], KT[i], QC[i], KC[i], VA[i]
                for nb in range(4):
                    n0 = nb * 512
                    avp = psum.tile([65, 512], F32, tag="sm", bufs=2,
                                    name=f"av_{i}_{nb}")
                    for mc in range(MC):
                        s2 = psum.tile([128, 512], F32, tag="qs", bufs=2,
                                       name=f"s2_{i}_{nb}_{mc}")
                        nc.tensor.matmul(s2[:], K[:, mc * 128:(mc + 1) * 128],
                                         Q[:, n0:n0 + 512],
                                         start=True, stop=False)
                        nc.tensor.matmul(s2[:], Kc[:, mc * 128:(mc + 1) * 128],
                                         Qc[:, n0:n0 + 512],
                                         start=False, stop=True)
                        at = attp.tile([128, 512], BF16, tag="att", bufs=4,
                                       name=f"at_{i}_{nb}_{mc}")
                        nc.scalar.activation(at[:], s2[:], ACT.Exp,
                                             bias=mbias_sb[:, j * MC + mc:j * MC + mc + 1],
                                             scale=1.0)
                        nc.tensor.matmul(avp[:], Va[:, mc, :], at[:],
                                         start=(mc == 0), stop=(mc == MC - 1))
                    # reciprocal is ~8 cyc/elem/lane on DVE; spread the 512
                    # values over 32 partitions via a tiny SBUF->SBUF DMA
                    lrow = aux.tile([1, 512], F32, tag="lrow", name=f"lr_{i}_{nb}")
                    nc.any.tensor_copy(lrow[0:1, :], avp[64:65, :])
                    l32 = aux.tile([32, 16], F32, tag="l32", name=f"l32_{i}_{nb}")
                    nc.sync.dma_start(l32[:, :], lrow[0:1, :])
                    r32 = aux.tile([32, 16], F32, tag="r32", name=f"r32_{i}_{nb}")
                    nc.vector.reciprocal(r32[:], l32[:])
                    linv = aux.tile([1, 512], F32, tag="linv", name=f"li_{i}_{nb}")
                    nc.sync.dma_start(linv[0:1, :], r32[:, :])
                    lb = aux.tile([64, 512], F32, tag="lb", name=f"lb_{i}_{nb}")
                    nc.gpsimd.partition_broadcast(lb[:], linv[0:1, :])
                    gt = aux.tile([64, 512], BF16, tag="gt", name=f"gt_{i}_{nb}")
                    nc.vector.tensor_mul(gt[:], avp[0:64, :], lb[:])
                    s = b * 4 + nb
                    nc.sync.dma_start(ag_in[s, 64 * j:64 * j + 64, :], gt[:])

            emit_pass1(0)
            for i in range(1, NBH):
                emit_pass1(i)
                emit_pass2(i - 1)
            emit_pass2(NBH - 1)

            # ---------- collective + output projection ----------
            nc.gpsimd.collective_compute(
                "AllToAll", ALU.bypass,
                replica_groups=[list(range(NCORES))],
                ins=[ag_in.opt()],
                outs=[ag_out.opt()],
            )
            gt_in = attp.tile([128, CCH, NSH], BF16, tag="gtin", bufs=1)
            nc.sync.dma_start(gt_in[:, :, :],
                              ag_out[:, :, :].rearrange("c p f -> p c f"))
            for et in range(CCH):
                yps = psum.tile([128, 512], F32, tag="qs", bufs=2, name=f"y{et}")
                for c in range(CCH):
                    nc.tensor.matmul(yps[:], wp_sb[:, c, et * 128:(et + 1) * 128],
                                     gt_in[:, c, :],
                                     start=(c == 0), stop=(c == CCH - 1))
                ysb = aux.tile([128, 512], F32, tag="y", name=f"ysb{et}")
                nc.scalar.activation(ysb[:], yps[:], ACT.Identity,
                                     bias=bp_sb[:, et:et + 1], scale=1.0)
                nc.sync.dma_start(out_t[et * 128:(et + 1) * 128, :], ysb[:])

    nc.compile()
    return nc


def _get_nc():
    global _compiled
    if _compiled is None:
        _compiled = _build()
    return _compiled


def _alibi_slopes():
    x = (2 ** 8) ** (1.0 / H)
    return np.array([1.0 / x ** (i + 1) for i in range(H)], dtype=np.float64)


def _chunked(a):
    """[C, F] -> [128, CCH, F] (partition, c-chunk, free)."""
    Cdim, F = a.shape
    return np.ascontiguousarray(a.reshape(CCH, 128, F).transpose(1, 0, 2))


def _split(a):
    hi = a.astype(BF)
    lo = (a - hi.astype(np.float32)).astype(BF)
    return hi, lo


def _make_in_maps(x, Wq, Wk, Wv, Wp, bp):
    x = np.asarray(x, dtype=np.float32)
    xT = np.ascontiguousarray(x.reshape(BN, C).T)          # [C, BN]
    xch = _chunked(xT)
    xch_hi, xch_lo = _split(xch)

    slopes = _alibi_slopes()
    n_arr = np.arange(N, dtype=np.float64)
    p_arr = np.arange(128, dtype=np.float64)

    wp_ch = _chunked(np.ascontiguousarray(np.asarray(Wp, np.float32).T)).astype(BF)
    bp_tile = np.ascontiguousarray(
        np.asarray(bp, np.float32).reshape(CCH, 128).T)
    identity = np.eye(128, dtype=np.float32)

    in_maps = []
    for core in range(NCORES):
        e0 = core * 128
        wqT = np.ascontiguousarray((8.0 * np.asarray(Wq, np.float32)[e0:e0 + 128]).T)
        wkT = np.ascontiguousarray(np.asarray(Wk, np.float32)[e0:e0 + 128].T)
        wvT = np.ascontiguousarray(np.asarray(Wv, np.float32)[e0:e0 + 128].T)
        wq_h, wq_l = _split(_chunked(wqT))
        wk_h, wk_l = _split(_chunked(wkT))

        s = slopes[core * HL: core * HL + HL]               # [HL]
        qaug = np.zeros((HL, 3, N), dtype=BF)
        kaug = np.zeros((HL, 3, N), dtype=BF)
        for j in range(HL):
            qaug[j, 0] = (-s[j] * n_arr).astype(BF)   # -slope*n
            qaug[j, 1] = 0.0                          # -M placeholder
            qaug[j, 2] = 1.0
            kaug[j, 0] = 1.0
            kaug[j, 1] = 1.0
            kaug[j, 2] = (s[j] * n_arr).astype(BF)    # +slope*m
        # pass2's K=128 main matmul already adds bf16(slope*m) via k row 66;
        # the exp bias supplies only the fp32 residual so the total is exact
        mb = np.zeros((128, HL * MC), dtype=np.float32)
        for j in range(HL):
            for c in range(MC):
                exact = (s[j] * (128 * c + p_arr)).astype(np.float32)
                mb[:, j * MC + c] = exact - exact.astype(BF).astype(np.float32)

        in_maps.append({
            "x_hi": xch_hi, "x_lo": xch_lo,
            "wq_hi": wq_h, "wq_lo": wq_l,
            "wk_hi": wk_h, "wk_lo": wk_l,
            "wv": _chunked(wvT).astype(BF),
            "wp": wp_ch, "bp_t": bp_tile,
            "qaug": qaug, "kaug": kaug, "mbias": mb,
            "ident": identity,
        })
    return in_maps


def run(x, Wq, Wk, Wv, Wp, bp, trace=False, tmpdir=None):
    nc = _get_nc()
    in_maps = _make_in_maps(x, Wq, Wk, Wv, Wp, bp)
    kwargs = {}
    if trace:
        kwargs = {"trace": True, "tmpdir": tmpdir}
    res = run_bass_kernel_spmd(nc, in_maps, core_ids=list(range(NCORES)), **kwargs)
    yT = np.concatenate([res.results[i]["out"] for i in range(NCORES)], axis=1)
    out = np.ascontiguousarray(yT.T).reshape(B, N, C).astype(np.float32)
    return out, res


def kernel(x, Wq, Wk, Wv, Wp, bp):
    out, _ = run(x, Wq, Wk, Wv, Wp, bp)
    return out
